# revision 16
# baseline (speedup 1.0000x reference)
"""Trainium2 Bass kernel for nn_MultiHeadAttention (B=16 heads, S=2048, D=1024, DH=64).

Sharding: 2 heads per core across 8 cores (head-parallel). Per core, the two
heads are processed in LOCKSTEP so every PE op is a concurrent tile-pair:
  - host pre-transposes+casts Q/K/V slices to bf16 chunk layout [2,128,8,S].
  - projections: col-tiled cross-head pairs (k0,k1), (q0,q1), (v0,v1) sharing
    one PSUM bank -> kT2/qT2/vT2 [128,S] with head0 in rows 0-63, head1 in
    rows 64-127 (exactly the layout the paired score matmuls need; no dups).
  - scores: row-tiled cross-head pairs (K=64 each) writing one [128,1024]
    PSUM tile (h0 | h1); one exp ACT per pair -> ex2 bf16 [128,1024].
  - AV: per-head accumulation chains with ones-column (row 64 = softmax
    denominator); AV emission deferred until v_aug ready (exp starts early).
  - normalize: reciprocal + col-paired ones-broadcast matmul + DVE muls.
  - per-(head,sq-half) AllGather of cc [64,1024] bf16; final GEMM
    yT_slice[128,S] = Wo_perm_slice @ cc + bo (column-sharded).
Host unshard: stack yT slices -> [1024,S] -> transpose -> [S,1024].
"""
import sys, os
sys.path.insert(0, '/opt/trn_rl_repo')
import numpy as np

_ABL = os.environ.get("KABL", "")

B = 16        # total heads
S = 2048
D = 1024
DH = 64
N_CORES = 8
HPC = B // N_CORES          # heads per core = 2
MS = D // N_CORES           # output column slice per core = 128

_runner = None


def _split_excess_waits(nc, mybir):
    """walrus in this env supports only ONE sync-wait command per instruction;
    hoist extra waits onto preceding single-wait NOPs on the same engine."""
    for f in nc.m.functions:
        for blk in f.blocks:
            new_list = []
            changed = False
            for ins in blk.instructions:
                si = ins.sync_info
                if si is not None and si.on_wait and len(si.on_wait) > 1:
                    waits = list(si.on_wait)
                    extra, keep = waits[:-1], waits[-1:]
                    for ci, w in enumerate(extra):
                        nop = mybir.InstNoOp(name=f"{ins.name}_wsplit_{ci}", ins=[], outs=[])
                        nop.engine = ins.engine
                        nop.sync_info = mybir.SyncInfo(on_wait=[w], on_update=[])
                        new_list.append(nop)
                    ins.sync_info = mybir.SyncInfo(on_wait=keep, on_update=list(si.on_update))
                    changed = True
                new_list.append(ins)
            if changed:
                blk.instructions = new_list


def _hoist_pair_ldws(nc, mybir):
    """Reorder [LDW1, MM1, LDW2, MM2] -> [LDW1, LDW2, MM1, MM2] when the two
    matmuls use disjoint PE-array regions (different row groups or col
    groups), letting the hardware run them as concurrent tiles. Safe because
    LDW2 writes array cells MM1 does not read, and MM order is unchanged."""

    def prange(ap):
        # (base_partition, count) from a physical access pattern
        try:
            stride, cnt = ap.ap[0]
            base = ap.offset // stride if stride else 0
            return int(base), int(cnt)
        except Exception:
            return None

    def disjoint(a, b):
        if a is None or b is None:
            return False
        return a[0] + a[1] <= b[0] or b[0] + b[1] <= a[0]

    for f in nc.m.functions:
        for blk in f.blocks:
            insts = blk.instructions
            pe_idx = [i for i, ins in enumerate(insts)
                      if getattr(ins, 'engine', None) == mybir.EngineType.PE]
            order = list(range(len(insts)))
            i = 0
            changed = False
            while i + 3 < len(pe_idx):
                i0, i1, i2, i3 = pe_idx[i], pe_idx[i + 1], pe_idx[i + 2], pe_idx[i + 3]
                a, b, c, d = insts[i0], insts[i1], insts[i2], insts[i3]
                if (isinstance(a, mybir.InstLdweights) and isinstance(b, mybir.InstMatmult)
                        and isinstance(c, mybir.InstLdweights) and isinstance(d, mybir.InstMatmult)
                        and i2 == i1 + 1  # LDW2 directly follows MM1
                        and not (c.sync_info and c.sync_info.on_wait)):
                    # row groups: stationary partition range; col groups: out range
                    rows1 = prange(b.ins[1]) if len(b.ins) > 1 else None
                    rows2 = prange(d.ins[1]) if len(d.ins) > 1 else None
                    cols1 = prange(b.outs[0]) if b.outs else None
                    cols2 = prange(d.outs[0]) if d.outs else None
                    if disjoint(rows1, rows2) or disjoint(cols1, cols2):
                        order[i1], order[i2] = order[i2], order[i1]
                        changed = True
                        i += 4
                        continue
                i += 2 if isinstance(a, mybir.InstLdweights) else 1
            if changed:
                blk.instructions = [insts[j] for j in order]


def build_nc(repeat=1, with_tail=True):
    """Build the per-core Bass program. repeat>1 wraps the compute body in a
    hardware loop (bench mode); the collective + final GEMM stay outside it."""
    import concourse.bass as bass
    import concourse.mybir as mybir
    import concourse.tile as tile
    from concourse.masks import make_identity

    F32 = mybir.dt.float32
    BF16 = mybir.dt.bfloat16
    AF = mybir.ActivationFunctionType

    nc = bass.Bass()

    qt_ext = nc.declare_dram_parameter("qt", [HPC, 128, 8, S], BF16, isOutput=False)
    kt_ext = nc.declare_dram_parameter("kt", [HPC, 128, 8, S], BF16, isOutput=False)
    vt_ext = nc.declare_dram_parameter("vt", [HPC, 128, 8, S], BF16, isOutput=False)
    wq_ext = nc.declare_dram_parameter("wq_t", [128, 8, DH], BF16, isOutput=False)
    wk_ext = nc.declare_dram_parameter("wk_t", [128, 8, DH], BF16, isOutput=False)
    wv_ext = nc.declare_dram_parameter("wv_t", [128, 8, DH], BF16, isOutput=False)
    bq_ext = nc.declare_dram_parameter("bq2", [128, 1], F32, isOutput=False)   # [bq|bq]
    bk_ext = nc.declare_dram_parameter("bk2", [128, 1], F32, isOutput=False)
    bv_ext = nc.declare_dram_parameter("bv2", [128, 1], F32, isOutput=False)
    wo_ext = nc.declare_dram_parameter("wo_t", [128, 8, MS], BF16, isOutput=False)
    bo_ext = nc.declare_dram_parameter("bo_s", [MS, 1], F32, isOutput=False)
    y_ext = nc.declare_dram_parameter("y", [MS, S], F32, isOutput=True)

    # cc chunks per (head, sq-half)
    cc_in = [[nc.dram_tensor(f"cc_in{h}{hf}", [DH, 1024], BF16) for hf in range(2)]
             for h in range(HPC)]
    cc_out = [[nc.dram_tensor(f"cc_out{h}{hf}", [DH * N_CORES, 1024], BF16,
                              addr_space="Shared") for hf in range(2)]
              for h in range(HPC)]

    with tile.TileContext(nc) as tc:
        with (
            tc.tile_pool(name="consts", bufs=1) as consts,
            nc.allow_low_precision(reason="bf16 matmuls by design"),
        ):
            # ---- constants ----
            ident_f32 = consts.tile([128, 128], F32)
            make_identity(nc, ident_f32)
            ident_bf = consts.tile([128, 128], BF16)
            nc.vector.tensor_copy(ident_bf[:], ident_f32[:])
            ones_bf = consts.tile([1, DH], BF16)
            nc.vector.memset(ones_bf, 1.0)

            biases = {}
            for nm, ext in (("q", bq_ext), ("k", bk_ext), ("v", bv_ext)):
                t = consts.tile([128, 1], F32, tag=f"b{nm}", name=f"b{nm}")
                nc.sync.dma_start(out=t[:], in_=ext[:])
                biases[nm] = t
            bo_sb = consts.tile([MS, 1], F32)
            nc.sync.dma_start(out=bo_sb[:], in_=bo_ext[:])

            w_sb = {}
            for nm, ext in (("q", wq_ext), ("k", wk_ext), ("v", wv_ext)):
                t = consts.tile([128, 8, DH], BF16, tag=f"w{nm}", name=f"w{nm}")
                nc.sync.dma_start(out=t[:], in_=ext[:])
                w_sb[nm] = t
            wo_sb = consts.tile([128, 8, MS], BF16)
            nc.sync.dma_start(out=wo_sb[:], in_=wo_ext[:])

            cc_sbuf = consts.tile([HPC * DH, S], BF16)

            with (
                tc.tile_pool(name="inp", bufs=3) as in_pool,
                tc.tile_pool(name="qkT", bufs=2) as qkT_pool,
                tc.tile_pool(name="vaug", bufs=4) as vaug_pool,
                tc.tile_pool(name="expp", bufs=18) as ex_pool,
                tc.tile_pool(name="smal", bufs=4) as small_pool,
                tc.tile_pool(name="pj_ps", bufs=2, space="PSUM") as pj_ps_pool,
                tc.tile_pool(name="sc_ps", bufs=2, space="PSUM") as sc_ps_pool,
                tc.tile_pool(name="ot_ps", bufs=2, space="PSUM") as ot_ps_pool,
            ):
                def load_input(ext):
                    """Both heads of one tensor, chunk-interleaved DMAs."""
                    ta = in_pool.tile([128, 8, S], BF16, tag="in", name="in_a")
                    tb = in_pool.tile([128, 8, S], BF16, tag="in", name="in_b")
                    for ci in range(4):
                        nc.sync.dma_start(out=ta[:, 2 * ci:2 * ci + 2, :],
                                          in_=ext[0, :, 2 * ci:2 * ci + 2, :])
                        nc.sync.dma_start(out=tb[:, 2 * ci:2 * ci + 2, :],
                                          in_=ext[1, :, 2 * ci:2 * ci + 2, :])
                    return ta, tb

                def proj_pair_step(ta, tb, nm, dest2, nb, ps):
                    """One column-quarter of the cross-head projection pair."""
                    for c in range(1 if "noproj" in _ABL else 8):
                        # start clears has_written only for this instruction's
                        # partition range, so each col-tile half starts its own
                        # accumulation group on the shared bank.
                        nc.tensor.matmul(ps[0:DH, :], w_sb[nm][:, c, :],
                                         ta[:, c, nb * 512:(nb + 1) * 512],
                                         start=(c == 0), stop=(c == 7),
                                         skip_group_check=True)
                        nc.tensor.matmul(ps[DH:128, :], w_sb[nm][:, c, :],
                                         tb[:, c, nb * 512:(nb + 1) * 512],
                                         start=(c == 0), stop=(c == 7),
                                         skip_group_check=True)
                    nc.vector.tensor_scalar_add(
                        dest2[:, nb * 512:(nb + 1) * 512], ps[:], biases[nm])

                def emit_proj_pair(ta, tb, nm, dest2):
                    for nb in range(4):
                        ps = pj_ps_pool.tile([128, 512], F32, tag="pp", name="pp")
                        proj_pair_step(ta, tb, nm, dest2, nb, ps)

                def compute_body(_iv=None):
                    k0_t, k1_t = load_input(kt_ext)
                    q0_t, q1_t = load_input(qt_ext)
                    v0_t, v1_t = load_input(vt_ext)

                    qT2 = qkT_pool.tile([128, S], BF16, tag="qT", name="qT2")
                    kT2 = qkT_pool.tile([128, S], BF16, tag="kT", name="kT2")
                    vT2 = qkT_pool.tile([128, S], BF16, tag="vT", name="vT2")

                    emit_proj_pair(k0_t, k1_t, "k", kT2)
                    emit_proj_pair(q0_t, q1_t, "q", qT2)

                    # deferred v projection: woven into attention as the
                    # chunks arrive; vaug transposes lazily per-j in flush.
                    va_ref = [None]

                    def v_step(nbs):
                        for nb in nbs:
                            ps = pj_ps_pool.tile([128, 512], F32, tag="pp", name="ppv")
                            proj_pair_step(v0_t, v1_t, "v", vT2, nb, ps)

                    def v_finish():
                        va0 = vaug_pool.tile([128, 16, DH + 1], BF16, tag="vaug", name="va0")
                        va1 = vaug_pool.tile([128, 16, DH + 1], BF16, tag="vaug", name="va1")
                        nc.vector.memset(va0[:, :, DH:DH + 1], 1.0)
                        nc.vector.memset(va1[:, :, DH:DH + 1], 1.0)
                        va_ref[0] = (va0, va1)

                    weave = {4: lambda: v_step([0]), 9: lambda: v_step([1]),
                             15: lambda: v_step([2]), 21: lambda: v_step([3]),
                             22: v_finish}
                    va_done = set()

                    def emit_vaug_j(j):
                        # lazily transpose one v chunk for both heads (row-pair)
                        va0, va1 = va_ref[0]
                        tp0 = pj_ps_pool.tile([128, DH], BF16, tag="pp", name="tp0")
                        tp1 = pj_ps_pool.tile([128, DH], BF16, tag="pp", name="tp1")
                        nc.tensor.transpose(tp0[:], vT2[0:DH, j * 128:(j + 1) * 128],
                                            ident_bf[0:DH, 0:DH])
                        nc.tensor.transpose(tp1[:], vT2[DH:128, j * 128:(j + 1) * 128],
                                            ident_bf[DH:128, DH:128])
                        nc.vector.tensor_copy(va0[:, j, 0:DH], tp0[:])
                        nc.vector.tensor_copy(va1[:, j, 0:DH], tp1[:])
                        va_done.add(j)

                    # ---- attention: heads in lockstep, sq in quarters ----
                    pend = []
                    ots = {}

                    def normalize(sqq, ot0, ot1):
                        s0 = sqq * 512
                        recips = []
                        for h, ot in ((0, ot0), (1, ot1)):
                            rc = small_pool.tile([1, 512], BF16, tag="recip",
                                                 name=f"rc{h}")
                            nc.vector.reciprocal(rc[:], ot[DH:DH + 1, :])
                            recips.append(rc)
                        bc2 = sc_ps_pool.tile([128, 1024], F32, tag="sc", name="bc2")
                        nc.tensor.matmul(bc2[0:DH, 0:512], ones_bf[:], recips[0][:],
                                         start=True, stop=True)
                        nc.tensor.matmul(bc2[DH:128, 0:512], ones_bf[:], recips[1][:],
                                         start=True, stop=True)
                        bc_sb = small_pool.tile([128, 512], F32, tag="bcsb", name="bcsb")
                        nc.vector.tensor_copy(bc_sb[:], bc2[:, 0:512])
                        nc.vector.tensor_mul(cc_sbuf[0:DH, s0:s0 + 512],
                                             ot0[0:DH, :], bc_sb[0:DH, :])
                        nc.vector.tensor_mul(cc_sbuf[DH:128, s0:s0 + 512],
                                             ot1[0:DH, :], bc_sb[DH:128, :])
                        if sqq % 2 == 1:
                            hf = sqq // 2
                            sl = slice(hf * 1024, (hf + 1) * 1024)
                            nc.scalar.dma_start(out=cc_in[0][hf][:, :],
                                                in_=cc_sbuf[0:DH, sl])
                            nc.scalar.dma_start(out=cc_in[1][hf][:, :],
                                                in_=cc_sbuf[DH:128, sl])

                    def flush(budget):
                        va0, va1 = va_ref[0]
                        while pend and budget > 0:
                            sqq, j, ex2 = pend.pop(0)
                            if j not in va_done:
                                emit_vaug_j(j)
                            if sqq not in ots:
                                ots[sqq] = (
                                    ot_ps_pool.tile([DH + 1, 512], F32, tag="ot", name="ot0"),
                                    ot_ps_pool.tile([DH + 1, 512], F32, tag="ot", name="ot1"),
                                )
                            ot0, ot1 = ots[sqq]
                            if "noav" in _ABL:
                                if j == 0:
                                    nc.vector.memset(ot0[0:1, 0:2], 1.0)
                                    nc.vector.memset(ot1[0:1, 0:2], 1.0)
                            else:
                                nc.tensor.matmul(ot0[:, :], va0[:, j, :], ex2[:, 0:512],
                                                 start=(j == 0), stop=(j == 15),
                                                 skip_group_check=True)
                                nc.tensor.matmul(ot1[:, :], va1[:, j, :], ex2[:, 512:1024],
                                                 start=(j == 0), stop=(j == 15),
                                                 skip_group_check=True)
                            if j == 15:
                                normalize(sqq, ot0, ot1)
                            budget -= 1

                    for sqq in range(4):
                        s0 = sqq * 512
                        for j in range(16):
                            g = sqq * 16 + j
                            sc2 = sc_ps_pool.tile([128, 1024], F32, tag="sc", name="sc2")
                            if "noscores" in _ABL:
                                nc.vector.memset(sc2[0:1, 0:2], 0.001)
                            else:
                                nc.tensor.matmul(sc2[:, 0:512],
                                                 kT2[0:DH, j * 128:(j + 1) * 128],
                                                 qT2[0:DH, s0:s0 + 512],
                                                 start=True, stop=True)
                                nc.tensor.matmul(sc2[:, 512:1024],
                                                 kT2[DH:128, j * 128:(j + 1) * 128],
                                                 qT2[DH:128, s0:s0 + 512],
                                                 start=True, stop=True)
                            ex2 = ex_pool.tile([128, 1024], BF16, tag="ex", name="ex2")
                            if "noact" in _ABL:
                                nc.vector.memset(ex2[0:1, 0:2], 0.001)
                            else:
                                nc.scalar.activation(ex2[:], sc2[:], AF.Exp)
                            pend.append((sqq, j, ex2))
                            w = weave.pop(g, None)
                            if w is not None:
                                w()
                            if va_ref[0] is not None:
                                flush(3)
                    flush(len(pend))

                if repeat == 1:
                    compute_body()
                else:
                    with tc.For_i(0, repeat, 1) as iv:
                        compute_body(iv)

            if with_tail:
                for hf in range(2):
                    for h in range(HPC):
                        nc.gpsimd.collective_compute(
                            "AllGather", mybir.AluOpType.bypass,
                            ins=[cc_in[h][hf][:]], outs=[cc_out[h][hf][:]],
                            replica_groups=[list(range(N_CORES))],
                        )
                with (
                    tc.tile_pool(name="ccf", bufs=4) as ccf_pool,
                    tc.tile_pool(name="ysb", bufs=2) as y_pool,
                    tc.tile_pool(name="y_ps", bufs=2, space="PSUM") as y_ps_pool,
                ):
                    for hf in range(2):
                        yt = y_ps_pool.tile([MS, 1024], F32, tag="yt", name=f"yt{hf}")
                        for g in range(8):
                            h, gc = divmod(g, 4)
                            cf = ccf_pool.tile([128, 1024], BF16, tag="ccf", name="ccf")
                            nc.sync.dma_start(
                                out=cf[:], in_=cc_out[h][hf][gc * 128:(gc + 1) * 128, :])
                            for sb in range(2):
                                nc.tensor.matmul(yt[:, sb * 512:(sb + 1) * 512],
                                                 wo_sb[:, g, :],
                                                 cf[:, sb * 512:(sb + 1) * 512],
                                                 start=(g == 0), stop=(g == 7))
                        for sb in range(2):
                            ysb = y_pool.tile([MS, 512], F32, tag="ysb", name="ysb")
                            nc.vector.tensor_scalar_add(
                                ysb[:], yt[:, sb * 512:(sb + 1) * 512], bo_sb[:])
                            nc.sync.dma_start(
                                out=y_ext[:, hf * 1024 + sb * 512:hf * 1024 + (sb + 1) * 512],
                                in_=ysb[:])

    _hoist_pair_ldws(nc, mybir)
    _split_excess_waits(nc, mybir)
    return nc


class SpmdRunner:
    """Compile once; execute repeatedly (mirrors bass2jax.run_bass_via_pjrt)."""

    def __init__(self, nc, n_cores):
        import jax
        import concourse.mybir as mybir
        from concourse.bass2jax import _bass_exec_p, partition_id_tensor, install_neuronx_cc_hook
        from jax.sharding import Mesh, PartitionSpec
        from jax.experimental.shard_map import shard_map

        install_neuronx_cc_hook()
        self.jax = jax
        self.n_cores = n_cores
        partition_name = nc.partition_id_tensor.name if nc.partition_id_tensor else None
        in_names, out_names, out_avals, zero_outs = [], [], [], []
        for alloc in nc.m.functions[0].allocations:
            if not isinstance(alloc, mybir.MemoryLocationSet):
                continue
            name = alloc.memorylocations[0].name
            if alloc.kind == "ExternalInput":
                if name != partition_name:
                    in_names.append(name)
            elif alloc.kind == "ExternalOutput":
                out_names.append(name)
                shape = tuple(alloc.tensor_shape)
                dtype = mybir.dt.np(alloc.dtype)
                out_avals.append(jax.core.ShapedArray(shape, dtype))
                zero_outs.append(np.zeros(shape, dtype))
        self.n_params = len(in_names)
        self.in_names = list(in_names)
        self.out_names = out_names
        self.out_avals = out_avals
        self.zero_outs = zero_outs
        all_names = in_names + out_names
        if partition_name is not None:
            all_names.append(partition_name)

        def _body(*args):
            operands = list(args)
            if partition_name is not None:
                operands.append(partition_id_tensor())
            outs = _bass_exec_p.bind(
                *operands,
                out_avals=tuple(out_avals),
                in_names=tuple(all_names),
                out_names=tuple(out_names),
                lowering_input_output_aliases=(),
                sim_require_finite=True,
                sim_require_nnan=True,
                nc=nc,
            )
            return tuple(outs)

        devices = jax.devices()[:n_cores]
        self.mesh = Mesh(np.asarray(devices), ("core",))
        n_outs = len(out_avals)
        donate = tuple(range(self.n_params, self.n_params + n_outs))
        self.sharded = jax.jit(
            shard_map(
                _body, mesh=self.mesh,
                in_specs=(PartitionSpec("core"),) * (self.n_params + n_outs),
                out_specs=(PartitionSpec("core"),) * n_outs,
                check_rep=False,
            ),
            donate_argnums=donate, keep_unused=True,
        )

    def concat_inputs(self, in_maps):
        per_core = [[np.ascontiguousarray(m[name]) for name in self.in_names] for m in in_maps]
        return [
            np.concatenate([per_core[c][i] for c in range(self.n_cores)], axis=0)
            for i in range(self.n_params)
        ]

    def run(self, concat_in):
        concat_zeros = [
            np.zeros((self.n_cores * z.shape[0], *z.shape[1:]), z.dtype)
            for z in self.zero_outs
        ]
        out_arrs = self.sharded(*concat_in, *concat_zeros)
        self.jax.block_until_ready(out_arrs)
        return out_arrs

    def split_outputs(self, out_arrs):
        return [
            {
                name: np.asarray(out_arrs[i]).reshape(self.n_cores, *self.out_avals[i].shape)[c]
                for i, name in enumerate(self.out_names)
            }
            for c in range(self.n_cores)
        ]


def make_in_maps(Q, K, V, Wq, bq, Wk, bk, Wv, bv, Wo, bo):
    """Shard full inputs into per-core input maps (layout prep only)."""
    import ml_dtypes
    BF = ml_dtypes.bfloat16
    scale = np.float32(1.0 / np.sqrt(DH))

    def wprep(w):
        # [DH, D] fp32 -> [128, 8, DH] bf16 with [p, c, h] = w[h, c*128+p]
        return np.ascontiguousarray(
            np.asarray(w, np.float32).T.reshape(8, 128, DH).transpose(1, 0, 2)
        ).astype(BF)

    wq_t = wprep(Wq)
    wk_t = wprep(np.asarray(Wk, np.float32) * scale)
    wv_t = wprep(Wv)

    def b2(b, s=1.0):
        x = (np.asarray(b, np.float32) * s).reshape(DH, 1)
        return np.concatenate([x, x], axis=0)

    bq_c, bk_c, bv_c = b2(bq), b2(bk, scale), b2(bv)
    Wo = np.asarray(Wo, np.float32)
    bo = np.asarray(bo, np.float32)

    # cc_out row -> original concat index permutation (per sq-half the AG for
    # (h, hf) gathers rows = head-local h of each core; concat = head*64+dh)
    perm = np.empty(D, np.int64)
    r = np.arange(512)
    perm[:512] = (2 * (r // DH)) * DH + r % DH
    perm[512:] = (2 * (r // DH) + 1) * DH + r % DH

    def xprep(X, c):
        xb = np.asarray(X[c * HPC:(c + 1) * HPC], np.float32).astype(BF)   # [2,S,D]
        xt = xb.transpose(0, 2, 1)                                         # [2,D,S]
        return np.ascontiguousarray(
            xt.reshape(HPC, 8, 128, S).transpose(0, 2, 1, 3))

    in_maps = []
    for c in range(N_CORES):
        wo_slice = Wo[c * MS:(c + 1) * MS, :][:, perm]                     # [128, 1024]
        wo_t = np.ascontiguousarray(
            wo_slice.T.reshape(8, 128, MS).transpose(1, 0, 2)).astype(BF)  # [128,8,128]
        in_maps.append({
            "qt": xprep(Q, c),
            "kt": xprep(K, c),
            "vt": xprep(V, c),
            "wq_t": wq_t, "wk_t": wk_t, "wv_t": wv_t,
            "bq2": bq_c, "bk2": bk_c, "bv2": bv_c,
            "wo_t": wo_t,
            "bo_s": bo[c * MS:(c + 1) * MS].reshape(MS, 1),
        })
    return in_maps


def get_runner():
    global _runner
    if _runner is None:
        nc = build_nc()
        _runner = SpmdRunner(nc, N_CORES)
    return _runner


def kernel(**inputs):
    r = get_runner()
    in_maps = make_in_maps(**inputs)
    out = r.run(r.concat_inputs(in_maps))
    res = r.split_outputs(out)
    y_t = np.concatenate([res[c]["y"] for c in range(N_CORES)], axis=0)  # [D, S]
    return np.ascontiguousarray(y_t.T).astype(np.float32)                # [S, D]


# revision 17
# speedup vs baseline: 1.0453x; 1.0453x over previous
"""Trainium2 Bass kernel for nn_MultiHeadAttention (B=16 heads, S=2048, D=1024, DH=64).

Sharding: 2 heads per core across 8 cores (head-parallel). Per core, the two
heads are processed in LOCKSTEP so every PE op is a concurrent tile-pair:
  - host pre-transposes+casts Q/K/V slices to bf16 chunk layout [2,128,8,S].
  - projections: col-tiled cross-head pairs (k0,k1), (q0,q1), (v0,v1) sharing
    one PSUM bank -> kT2/qT2/vT2 [128,S] with head0 in rows 0-63, head1 in
    rows 64-127 (exactly the layout the paired score matmuls need; no dups).
  - scores: row-tiled cross-head pairs (K=64 each) writing one [128,1024]
    PSUM tile (h0 | h1); one exp ACT per pair -> ex2 bf16 [128,1024].
  - AV: per-head accumulation chains with ones-column (row 64 = softmax
    denominator); AV emission deferred until v_aug ready (exp starts early).
  - normalize: reciprocal + col-paired ones-broadcast matmul + DVE muls.
  - per-(head,sq-half) AllGather of cc [64,1024] bf16; final GEMM
    yT_slice[128,S] = Wo_perm_slice @ cc + bo (column-sharded).
Host unshard: stack yT slices -> [1024,S] -> transpose -> [S,1024].
"""
import sys, os
sys.path.insert(0, '/opt/trn_rl_repo')
import numpy as np

_ABL = os.environ.get("KABL", "")

B = 16        # total heads
S = 2048
D = 1024
DH = 64
N_CORES = 8
HPC = B // N_CORES          # heads per core = 2
MS = D // N_CORES           # output column slice per core = 128

_runner = None


def _split_excess_waits(nc, mybir):
    """walrus in this env supports only ONE sync-wait command per instruction;
    hoist extra waits onto preceding single-wait NOPs on the same engine."""
    for f in nc.m.functions:
        for blk in f.blocks:
            new_list = []
            changed = False
            for ins in blk.instructions:
                si = ins.sync_info
                if si is not None and si.on_wait and len(si.on_wait) > 1:
                    waits = list(si.on_wait)
                    extra, keep = waits[:-1], waits[-1:]
                    for ci, w in enumerate(extra):
                        nop = mybir.InstNoOp(name=f"{ins.name}_wsplit_{ci}", ins=[], outs=[])
                        nop.engine = ins.engine
                        nop.sync_info = mybir.SyncInfo(on_wait=[w], on_update=[])
                        new_list.append(nop)
                    ins.sync_info = mybir.SyncInfo(on_wait=keep, on_update=list(si.on_update))
                    changed = True
                new_list.append(ins)
            if changed:
                blk.instructions = new_list


def _hoist_pair_ldws(nc, mybir):
    """Reorder [LDW1, MM1, LDW2, MM2] -> [LDW1, LDW2, MM1, MM2] when the two
    matmuls use disjoint PE-array regions (different row groups or col
    groups), letting the hardware run them as concurrent tiles. Safe because
    LDW2 writes array cells MM1 does not read, and MM order is unchanged."""

    def prange(ap):
        # (base_partition, count) from a physical access pattern
        try:
            stride, cnt = ap.ap[0]
            base = ap.offset // stride if stride else 0
            return int(base), int(cnt)
        except Exception:
            return None

    def disjoint(a, b):
        if a is None or b is None:
            return False
        return a[0] + a[1] <= b[0] or b[0] + b[1] <= a[0]

    for f in nc.m.functions:
        for blk in f.blocks:
            insts = blk.instructions
            pe_idx = [i for i, ins in enumerate(insts)
                      if getattr(ins, 'engine', None) == mybir.EngineType.PE]
            order = list(range(len(insts)))
            i = 0
            changed = False
            while i + 3 < len(pe_idx):
                i0, i1, i2, i3 = pe_idx[i], pe_idx[i + 1], pe_idx[i + 2], pe_idx[i + 3]
                a, b, c, d = insts[i0], insts[i1], insts[i2], insts[i3]
                if (isinstance(a, mybir.InstLdweights) and isinstance(b, mybir.InstMatmult)
                        and isinstance(c, mybir.InstLdweights) and isinstance(d, mybir.InstMatmult)
                        and i2 == i1 + 1  # LDW2 directly follows MM1
                        and not (c.sync_info and c.sync_info.on_wait)):
                    # row groups: stationary partition range; col groups: out range
                    rows1 = prange(b.ins[1]) if len(b.ins) > 1 else None
                    rows2 = prange(d.ins[1]) if len(d.ins) > 1 else None
                    cols1 = prange(b.outs[0]) if b.outs else None
                    cols2 = prange(d.outs[0]) if d.outs else None
                    if disjoint(rows1, rows2) or disjoint(cols1, cols2):
                        order[i1], order[i2] = order[i2], order[i1]
                        changed = True
                        i += 4
                        continue
                i += 2 if isinstance(a, mybir.InstLdweights) else 1
            if changed:
                blk.instructions = [insts[j] for j in order]


def build_nc(repeat=1, with_tail=True):
    """Build the per-core Bass program. repeat>1 wraps the compute body in a
    hardware loop (bench mode); the collective + final GEMM stay outside it."""
    import concourse.bass as bass
    import concourse.mybir as mybir
    import concourse.tile as tile
    from concourse.masks import make_identity

    F32 = mybir.dt.float32
    BF16 = mybir.dt.bfloat16
    AF = mybir.ActivationFunctionType

    nc = bass.Bass()

    qt_ext = nc.declare_dram_parameter("qt", [HPC, 128, 8, S], BF16, isOutput=False)
    kt_ext = nc.declare_dram_parameter("kt", [HPC, 128, 8, S], BF16, isOutput=False)
    vt_ext = nc.declare_dram_parameter("vt", [HPC, 128, 8, S], BF16, isOutput=False)
    wq_ext = nc.declare_dram_parameter("wq_t", [128, 8, DH], BF16, isOutput=False)
    wk_ext = nc.declare_dram_parameter("wk_t", [128, 8, DH], BF16, isOutput=False)
    wv_ext = nc.declare_dram_parameter("wv_t", [128, 8, DH], BF16, isOutput=False)
    bq_ext = nc.declare_dram_parameter("bq2", [128, 1], F32, isOutput=False)   # [bq|bq]
    bk_ext = nc.declare_dram_parameter("bk2", [128, 1], F32, isOutput=False)
    bv_ext = nc.declare_dram_parameter("bv2", [128, 1], F32, isOutput=False)
    wo_ext = nc.declare_dram_parameter("wo_t", [128, 8, MS], BF16, isOutput=False)
    bo_ext = nc.declare_dram_parameter("bo_s", [MS, 1], F32, isOutput=False)
    y_ext = nc.declare_dram_parameter("y", [MS, S], F32, isOutput=True)

    # cc chunks per (head, sq-half)
    cc_in = [[nc.dram_tensor(f"cc_in{h}{hf}", [DH, 1024], BF16) for hf in range(2)]
             for h in range(HPC)]
    cc_out = [[nc.dram_tensor(f"cc_out{h}{hf}", [DH * N_CORES, 1024], BF16,
                              addr_space="Shared") for hf in range(2)]
              for h in range(HPC)]

    with tile.TileContext(nc) as tc:
        with (
            tc.tile_pool(name="consts", bufs=1) as consts,
            nc.allow_low_precision(reason="bf16 matmuls by design"),
        ):
            # ---- constants ----
            ident_f32 = consts.tile([128, 128], F32)
            make_identity(nc, ident_f32)
            ident_bf = consts.tile([128, 128], BF16)
            nc.vector.tensor_copy(ident_bf[:], ident_f32[:])
            ones_bf = consts.tile([1, DH], BF16)
            nc.vector.memset(ones_bf, 1.0)
            ones128 = consts.tile([128, 1], BF16)
            nc.vector.memset(ones128, 1.0)

            biases = {}
            for nm, ext in (("q", bq_ext), ("k", bk_ext), ("v", bv_ext)):
                t = consts.tile([128, 1], F32, tag=f"b{nm}", name=f"b{nm}")
                nc.sync.dma_start(out=t[:], in_=ext[:])
                biases[nm] = t
            bo_sb = consts.tile([MS, 1], F32)
            nc.sync.dma_start(out=bo_sb[:], in_=bo_ext[:])

            w_sb = {}
            for nm, ext in (("q", wq_ext), ("k", wk_ext), ("v", wv_ext)):
                t = consts.tile([128, 8, DH], BF16, tag=f"w{nm}", name=f"w{nm}")
                nc.sync.dma_start(out=t[:], in_=ext[:])
                w_sb[nm] = t
            wo_sb = consts.tile([128, 8, MS], BF16)
            nc.sync.dma_start(out=wo_sb[:], in_=wo_ext[:])

            cc_sbuf = consts.tile([HPC * DH, S], BF16)

            with (
                tc.tile_pool(name="inp", bufs=3) as in_pool,
                tc.tile_pool(name="qkT", bufs=2) as qkT_pool,
                tc.tile_pool(name="vaug", bufs=4) as vaug_pool,
                tc.tile_pool(name="expp", bufs=18) as ex_pool,
                tc.tile_pool(name="smal", bufs=4) as small_pool,
                tc.tile_pool(name="denp", bufs=4) as den_pool,
                tc.tile_pool(name="pj_ps", bufs=2, space="PSUM") as pj_ps_pool,
                tc.tile_pool(name="sc_ps", bufs=2, space="PSUM") as sc_ps_pool,
                tc.tile_pool(name="ot_ps", bufs=2, space="PSUM") as ot_ps_pool,
            ):
                def load_input(ext):
                    """Both heads of one tensor, chunk-interleaved DMAs."""
                    ta = in_pool.tile([128, 8, S], BF16, tag="in", name="in_a")
                    tb = in_pool.tile([128, 8, S], BF16, tag="in", name="in_b")
                    for ci in range(4):
                        nc.sync.dma_start(out=ta[:, 2 * ci:2 * ci + 2, :],
                                          in_=ext[0, :, 2 * ci:2 * ci + 2, :])
                        nc.sync.dma_start(out=tb[:, 2 * ci:2 * ci + 2, :],
                                          in_=ext[1, :, 2 * ci:2 * ci + 2, :])
                    return ta, tb

                def proj_pair_step(ta, tb, nm, dest2, nb, ps):
                    """One column-quarter of the cross-head projection pair."""
                    for c in range(1 if "noproj" in _ABL else 8):
                        # start clears has_written only for this instruction's
                        # partition range, so each col-tile half starts its own
                        # accumulation group on the shared bank.
                        nc.tensor.matmul(ps[0:DH, :], w_sb[nm][:, c, :],
                                         ta[:, c, nb * 512:(nb + 1) * 512],
                                         start=(c == 0), stop=(c == 7),
                                         skip_group_check=True)
                        nc.tensor.matmul(ps[DH:128, :], w_sb[nm][:, c, :],
                                         tb[:, c, nb * 512:(nb + 1) * 512],
                                         start=(c == 0), stop=(c == 7),
                                         skip_group_check=True)
                    nc.vector.tensor_scalar_add(
                        dest2[:, nb * 512:(nb + 1) * 512], ps[:], biases[nm])

                def emit_proj_pair(ta, tb, nm, dest2):
                    for nb in range(4):
                        ps = pj_ps_pool.tile([128, 512], F32, tag="pp", name="pp")
                        proj_pair_step(ta, tb, nm, dest2, nb, ps)

                def compute_body(_iv=None):
                    k0_t, k1_t = load_input(kt_ext)
                    q0_t, q1_t = load_input(qt_ext)
                    v0_t, v1_t = load_input(vt_ext)

                    qT2 = qkT_pool.tile([128, S], BF16, tag="qT", name="qT2")
                    kT2 = qkT_pool.tile([128, S], BF16, tag="kT", name="kT2")
                    vT2 = qkT_pool.tile([128, S], BF16, tag="vT", name="vT2")

                    emit_proj_pair(k0_t, k1_t, "k", kT2)
                    emit_proj_pair(q0_t, q1_t, "q", qT2)

                    # deferred v projection: woven into attention as the
                    # chunks arrive; vaug transposes lazily per-j in flush.
                    va_ref = [None]

                    def v_step(nbs):
                        for nb in nbs:
                            ps = pj_ps_pool.tile([128, 512], F32, tag="pp", name="ppv")
                            proj_pair_step(v0_t, v1_t, "v", vT2, nb, ps)

                    def v_finish():
                        va0 = vaug_pool.tile([128, 16, DH], BF16, tag="vaug", name="va0")
                        va1 = vaug_pool.tile([128, 16, DH], BF16, tag="vaug", name="va1")
                        va_ref[0] = (va0, va1)

                    weave = {4: lambda: v_step([0]), 9: lambda: v_step([1]),
                             15: lambda: v_step([2]), 21: lambda: v_step([3]),
                             22: v_finish}
                    va_done = set()

                    def emit_vaug_j(j):
                        # lazily transpose one v chunk for both heads (row-pair)
                        va0, va1 = va_ref[0]
                        tp0 = pj_ps_pool.tile([128, DH], BF16, tag="pp", name="tp0")
                        tp1 = pj_ps_pool.tile([128, DH], BF16, tag="pp", name="tp1")
                        nc.tensor.transpose(tp0[:], vT2[0:DH, j * 128:(j + 1) * 128],
                                            ident_bf[0:DH, 0:DH])
                        nc.tensor.transpose(tp1[:], vT2[DH:128, j * 128:(j + 1) * 128],
                                            ident_bf[DH:128, DH:128])
                        nc.vector.tensor_copy(va0[:, j, 0:DH], tp0[:])
                        nc.vector.tensor_copy(va1[:, j, 0:DH], tp1[:])
                        va_done.add(j)

                    # ---- attention: heads in lockstep, sq in quarters ----
                    pend = []
                    ots = {}

                    def normalize(sqq, ot2, den0, den1):
                        s0 = sqq * 512
                        red = sc_ps_pool.tile([128, 1024], F32, tag="sc", name="red")
                        nc.tensor.matmul(red[0:1, 0:512], ones128[:], den0[:],
                                         start=True, stop=True)
                        nc.tensor.matmul(red[0:1, 512:1024], ones128[:], den1[:],
                                         start=True, stop=True)
                        rc2 = small_pool.tile([1, 1024], BF16, tag="recip", name="rc2")
                        nc.vector.reciprocal(rc2[:], red[0:1, :])
                        bc2 = sc_ps_pool.tile([128, 1024], F32, tag="sc", name="bc2")
                        nc.tensor.matmul(bc2[0:DH, 0:512], ones_bf[:], rc2[:, 0:512],
                                         start=True, stop=True)
                        nc.tensor.matmul(bc2[DH:128, 0:512], ones_bf[:], rc2[:, 512:1024],
                                         start=True, stop=True)
                        bc_sb = small_pool.tile([128, 512], F32, tag="bcsb", name="bcsb")
                        nc.vector.tensor_copy(bc_sb[:], bc2[:, 0:512])
                        nc.vector.tensor_mul(cc_sbuf[:, s0:s0 + 512], ot2[:, :], bc_sb[:])
                        if sqq % 2 == 1:
                            hf = sqq // 2
                            sl = slice(hf * 1024, (hf + 1) * 1024)
                            nc.scalar.dma_start(out=cc_in[0][hf][:, :],
                                                in_=cc_sbuf[0:DH, sl])
                            nc.scalar.dma_start(out=cc_in[1][hf][:, :],
                                                in_=cc_sbuf[DH:128, sl])

                    def flush(budget):
                        va0, va1 = va_ref[0]
                        while pend and budget > 0:
                            sqq, j, ex2 = pend.pop(0)
                            if j not in va_done:
                                emit_vaug_j(j)
                            if sqq not in ots:
                                ots[sqq] = (
                                    ot_ps_pool.tile([128, 512], F32, tag="ot", name="ot2"),
                                    den_pool.tile([128, 512], BF16, tag="dn", name="den0"),
                                    den_pool.tile([128, 512], BF16, tag="dn", name="den1"),
                                )
                            ot2, den0, den1 = ots[sqq]
                            if "noav" in _ABL:
                                if j == 0:
                                    nc.vector.memset(ot2[0:1, 0:2], 1.0)
                                    nc.vector.memset(den0[0:1, 0:2], 1.0)
                                    nc.vector.memset(den1[0:1, 0:2], 1.0)
                            else:
                                nc.tensor.matmul(ot2[0:DH, :], va0[:, j, :], ex2[:, 0:512],
                                                 start=(j == 0), stop=(j == 15),
                                                 skip_group_check=True)
                                nc.tensor.matmul(ot2[DH:128, :], va1[:, j, :], ex2[:, 512:1024],
                                                 start=(j == 0), stop=(j == 15),
                                                 skip_group_check=True)
                                if j == 0:
                                    nc.vector.tensor_copy(den0[:], ex2[:, 0:512])
                                    nc.vector.tensor_copy(den1[:], ex2[:, 512:1024])
                                else:
                                    nc.vector.tensor_add(den0[:], den0[:], ex2[:, 0:512])
                                    nc.vector.tensor_add(den1[:], den1[:], ex2[:, 512:1024])
                            if j == 15:
                                normalize(sqq, ot2, den0, den1)
                            budget -= 1

                    for sqq in range(4):
                        s0 = sqq * 512
                        for j in range(16):
                            g = sqq * 16 + j
                            sc2 = sc_ps_pool.tile([128, 1024], F32, tag="sc", name="sc2")
                            if "noscores" in _ABL:
                                nc.vector.memset(sc2[0:1, 0:2], 0.001)
                            else:
                                nc.tensor.matmul(sc2[:, 0:512],
                                                 kT2[0:DH, j * 128:(j + 1) * 128],
                                                 qT2[0:DH, s0:s0 + 512],
                                                 start=True, stop=True)
                                nc.tensor.matmul(sc2[:, 512:1024],
                                                 kT2[DH:128, j * 128:(j + 1) * 128],
                                                 qT2[DH:128, s0:s0 + 512],
                                                 start=True, stop=True)
                            ex2 = ex_pool.tile([128, 1024], BF16, tag="ex", name="ex2")
                            if "noact" in _ABL:
                                nc.vector.memset(ex2[0:1, 0:2], 0.001)
                            else:
                                nc.scalar.activation(ex2[:], sc2[:], AF.Exp)
                            pend.append((sqq, j, ex2))
                            w = weave.pop(g, None)
                            if w is not None:
                                w()
                            if va_ref[0] is not None:
                                flush(3)
                    flush(len(pend))

                if repeat == 1:
                    compute_body()
                else:
                    with tc.For_i(0, repeat, 1) as iv:
                        compute_body(iv)

            if with_tail:
                for hf in range(2):
                    for h in range(HPC):
                        nc.gpsimd.collective_compute(
                            "AllGather", mybir.AluOpType.bypass,
                            ins=[cc_in[h][hf][:]], outs=[cc_out[h][hf][:]],
                            replica_groups=[list(range(N_CORES))],
                        )
                with (
                    tc.tile_pool(name="ccf", bufs=4) as ccf_pool,
                    tc.tile_pool(name="ysb", bufs=2) as y_pool,
                    tc.tile_pool(name="y_ps", bufs=2, space="PSUM") as y_ps_pool,
                ):
                    for hf in range(2):
                        yt = y_ps_pool.tile([MS, 1024], F32, tag="yt", name=f"yt{hf}")
                        for g in range(8):
                            h, gc = divmod(g, 4)
                            cf = ccf_pool.tile([128, 1024], BF16, tag="ccf", name="ccf")
                            nc.sync.dma_start(
                                out=cf[:], in_=cc_out[h][hf][gc * 128:(gc + 1) * 128, :])
                            for sb in range(2):
                                nc.tensor.matmul(yt[:, sb * 512:(sb + 1) * 512],
                                                 wo_sb[:, g, :],
                                                 cf[:, sb * 512:(sb + 1) * 512],
                                                 start=(g == 0), stop=(g == 7))
                        for sb in range(2):
                            ysb = y_pool.tile([MS, 512], F32, tag="ysb", name="ysb")
                            nc.vector.tensor_scalar_add(
                                ysb[:], yt[:, sb * 512:(sb + 1) * 512], bo_sb[:])
                            nc.sync.dma_start(
                                out=y_ext[:, hf * 1024 + sb * 512:hf * 1024 + (sb + 1) * 512],
                                in_=ysb[:])

    _hoist_pair_ldws(nc, mybir)
    _split_excess_waits(nc, mybir)
    return nc


class SpmdRunner:
    """Compile once; execute repeatedly (mirrors bass2jax.run_bass_via_pjrt)."""

    def __init__(self, nc, n_cores):
        import jax
        import concourse.mybir as mybir
        from concourse.bass2jax import _bass_exec_p, partition_id_tensor, install_neuronx_cc_hook
        from jax.sharding import Mesh, PartitionSpec
        from jax.experimental.shard_map import shard_map

        install_neuronx_cc_hook()
        self.jax = jax
        self.n_cores = n_cores
        partition_name = nc.partition_id_tensor.name if nc.partition_id_tensor else None
        in_names, out_names, out_avals, zero_outs = [], [], [], []
        for alloc in nc.m.functions[0].allocations:
            if not isinstance(alloc, mybir.MemoryLocationSet):
                continue
            name = alloc.memorylocations[0].name
            if alloc.kind == "ExternalInput":
                if name != partition_name:
                    in_names.append(name)
            elif alloc.kind == "ExternalOutput":
                out_names.append(name)
                shape = tuple(alloc.tensor_shape)
                dtype = mybir.dt.np(alloc.dtype)
                out_avals.append(jax.core.ShapedArray(shape, dtype))
                zero_outs.append(np.zeros(shape, dtype))
        self.n_params = len(in_names)
        self.in_names = list(in_names)
        self.out_names = out_names
        self.out_avals = out_avals
        self.zero_outs = zero_outs
        all_names = in_names + out_names
        if partition_name is not None:
            all_names.append(partition_name)

        def _body(*args):
            operands = list(args)
            if partition_name is not None:
                operands.append(partition_id_tensor())
            outs = _bass_exec_p.bind(
                *operands,
                out_avals=tuple(out_avals),
                in_names=tuple(all_names),
                out_names=tuple(out_names),
                lowering_input_output_aliases=(),
                sim_require_finite=True,
                sim_require_nnan=True,
                nc=nc,
            )
            return tuple(outs)

        devices = jax.devices()[:n_cores]
        self.mesh = Mesh(np.asarray(devices), ("core",))
        n_outs = len(out_avals)
        donate = tuple(range(self.n_params, self.n_params + n_outs))
        self.sharded = jax.jit(
            shard_map(
                _body, mesh=self.mesh,
                in_specs=(PartitionSpec("core"),) * (self.n_params + n_outs),
                out_specs=(PartitionSpec("core"),) * n_outs,
                check_rep=False,
            ),
            donate_argnums=donate, keep_unused=True,
        )

    def concat_inputs(self, in_maps):
        per_core = [[np.ascontiguousarray(m[name]) for name in self.in_names] for m in in_maps]
        return [
            np.concatenate([per_core[c][i] for c in range(self.n_cores)], axis=0)
            for i in range(self.n_params)
        ]

    def run(self, concat_in):
        concat_zeros = [
            np.zeros((self.n_cores * z.shape[0], *z.shape[1:]), z.dtype)
            for z in self.zero_outs
        ]
        out_arrs = self.sharded(*concat_in, *concat_zeros)
        self.jax.block_until_ready(out_arrs)
        return out_arrs

    def split_outputs(self, out_arrs):
        return [
            {
                name: np.asarray(out_arrs[i]).reshape(self.n_cores, *self.out_avals[i].shape)[c]
                for i, name in enumerate(self.out_names)
            }
            for c in range(self.n_cores)
        ]


def make_in_maps(Q, K, V, Wq, bq, Wk, bk, Wv, bv, Wo, bo):
    """Shard full inputs into per-core input maps (layout prep only)."""
    import ml_dtypes
    BF = ml_dtypes.bfloat16
    scale = np.float32(1.0 / np.sqrt(DH))

    def wprep(w):
        # [DH, D] fp32 -> [128, 8, DH] bf16 with [p, c, h] = w[h, c*128+p]
        return np.ascontiguousarray(
            np.asarray(w, np.float32).T.reshape(8, 128, DH).transpose(1, 0, 2)
        ).astype(BF)

    wq_t = wprep(Wq)
    wk_t = wprep(np.asarray(Wk, np.float32) * scale)
    wv_t = wprep(Wv)

    def b2(b, s=1.0):
        x = (np.asarray(b, np.float32) * s).reshape(DH, 1)
        return np.concatenate([x, x], axis=0)

    bq_c, bk_c, bv_c = b2(bq), b2(bk, scale), b2(bv)
    Wo = np.asarray(Wo, np.float32)
    bo = np.asarray(bo, np.float32)

    # cc_out row -> original concat index permutation (per sq-half the AG for
    # (h, hf) gathers rows = head-local h of each core; concat = head*64+dh)
    perm = np.empty(D, np.int64)
    r = np.arange(512)
    perm[:512] = (2 * (r // DH)) * DH + r % DH
    perm[512:] = (2 * (r // DH) + 1) * DH + r % DH

    def xprep(X, c):
        xb = np.asarray(X[c * HPC:(c + 1) * HPC], np.float32).astype(BF)   # [2,S,D]
        xt = xb.transpose(0, 2, 1)                                         # [2,D,S]
        return np.ascontiguousarray(
            xt.reshape(HPC, 8, 128, S).transpose(0, 2, 1, 3))

    in_maps = []
    for c in range(N_CORES):
        wo_slice = Wo[c * MS:(c + 1) * MS, :][:, perm]                     # [128, 1024]
        wo_t = np.ascontiguousarray(
            wo_slice.T.reshape(8, 128, MS).transpose(1, 0, 2)).astype(BF)  # [128,8,128]
        in_maps.append({
            "qt": xprep(Q, c),
            "kt": xprep(K, c),
            "vt": xprep(V, c),
            "wq_t": wq_t, "wk_t": wk_t, "wv_t": wv_t,
            "bq2": bq_c, "bk2": bk_c, "bv2": bv_c,
            "wo_t": wo_t,
            "bo_s": bo[c * MS:(c + 1) * MS].reshape(MS, 1),
        })
    return in_maps


def get_runner():
    global _runner
    if _runner is None:
        nc = build_nc()
        _runner = SpmdRunner(nc, N_CORES)
    return _runner


def kernel(**inputs):
    r = get_runner()
    in_maps = make_in_maps(**inputs)
    out = r.run(r.concat_inputs(in_maps))
    res = r.split_outputs(out)
    y_t = np.concatenate([res[c]["y"] for c in range(N_CORES)], axis=0)  # [D, S]
    return np.ascontiguousarray(y_t.T).astype(np.float32)                # [S, D]
